# revision 47
# baseline (speedup 1.0000x reference)
"""Packed causal GQA attention (B=4 x S=1024, H=32, KVH=8, D=DV=128, fp32)
for 8 Trainium2 NeuronCores.

Sharding: tensor-parallel over KV heads. Core c owns kv head c and its GQA
group of 4 query heads (4c..4c+3). No cross-core communication.

Per-core pipeline, software-pipelined over 16 (b, h) units (front of unit
u+1 is emitted before the back of unit u so the PE never waits in-order
on the exp of scores it just produced):

  front: QK (fp16): S^T[k, q] = K^T.T @ Q^T on PE, causal column ranges,
    fp32 PSUM; P^T = Exp(SCALE*S^T) -> fp16 into a dense [128, kb, q]
    SBUF tile. k-blocks 0-4 exp on the ACT engine; k-blocks 5-7 use a
    Schraudolph bit-trick exp on the DVE (uint16(A*x+B) bit-viewed as
    fp16) to offload the ACT bottleneck. gpsimd affine_select masks the
    strictly-upper triangle of each diagonal block.

  back: PV with fused softmax denominator on PE: rhs = [V | 1] (129
    cols); for each q-block qb, psum[q, 0:128] = sum_kb P^T_kb(qb).T @
    V_kb and psum[q, 128] = l[q], accumulated over kb in a single-bank
    PSUM tile. This kills the separate ones-matmul (1/3 of all PE
    columns in the classic scheme) and yields the output in natural
    [q, dv] layout (no transposes anywhere). normalize = DVE
    reciprocal_approx_fast on l + per-partition tensor_scalar multiply;
    fp16 store.
"""

import numpy as np

import concourse.bacc as bacc
import concourse.tile as tile
from concourse import mybir, bass_utils

T = 4096          # packed tokens
SEQ = 1024        # per-sequence length
B = T // SEQ      # 4 sequences
H = 32            # query heads (total)
KVH = 8           # kv heads (total)
D = 128           # head size
DV = 128          # value head size
NCORES = 8
HPC = H // NCORES         # 4 query heads per core
NB = SEQ // 128           # 8 k-blocks per sequence
SCALE = 0.08838834764831845

F16 = mybir.dt.float16
F32 = mybir.dt.float32
U16 = mybir.dt.uint16

LN2 = 0.6931471805599453
A16 = 1024.0 / LN2        # fp16 Schraudolph slope
B16 = 15316.5             # 15*1024 - magic bias + trunc-rounding comp

DVE_EXP_KBS = (5, 6)      # k-blocks whose exp runs on DVE
ACT_TS_QBS = ()          # q-blocks whose normalize multiply runs on ACT

_BUILD_CACHE = {}


def _build_nc():
    nc = bacc.Bacc("TRN2", target_bir_lowering=False, debug=False,
                   num_devices=NCORES)
    # host-pretransposed, fp16: qT[h*128+d, t], kT[d, t]; v[t, dv]
    qt_dram = nc.dram_tensor("qT", [HPC * D, T], F16, kind="ExternalInput").ap()
    kt_dram = nc.dram_tensor("kT", [D, T], F16, kind="ExternalInput").ap()
    v_dram = nc.dram_tensor("v", [T, DV], F16, kind="ExternalInput").ap()
    # natural layout output: out[t, h*DV + dv] for this core's 4 heads
    out_dram = nc.dram_tensor("out", [T, HPC * DV], F16,
                              kind="ExternalOutput").ap()

    with tile.TileContext(nc) as tc:
        with tc.tile_pool(name="kv", bufs=3) as kv_pool, \
             tc.tile_pool(name="qts", bufs=12) as qt_pool, \
             tc.tile_pool(name="pt", bufs=4) as pt_pool, \
             tc.tile_pool(name="work", bufs=3) as work, \
             tc.tile_pool(name="pp_s", bufs=3, space="PSUM") as pp_s, \
             tc.tile_pool(name="pp_v", bufs=2, space="PSUM") as pp_v:

            per_b = {}   # b -> (kt, vext, [qt0..qt3])

            def emit_loads(b):
                cols = slice(b * SEQ, (b + 1) * SEQ)
                rows = slice(b * SEQ, (b + 1) * SEQ)
                kt = kv_pool.tile([128, NB, 128], F16, tag="kt")
                ktv = kt_dram[:, cols].rearrange("d (nb t) -> d nb t", t=128)
                qts = []
                qtvs = []
                for h in range(HPC):
                    qt = qt_pool.tile([128, NB, 128], F16, tag="qt")
                    qts.append(qt)
                    qtvs.append(qt_dram[h * D:(h + 1) * D, cols].rearrange(
                        "d (nb t) -> d nb t", t=128))
                vext = kv_pool.tile([128, NB, DV + 1], F16, tag="v")
                vv = v_dram[rows, :].rearrange("(nb p) d -> p nb d", p=128)
                if b == 0:
                    # critical path: dispatch in need-order through the two
                    # hardware DGE queues (sync + scalar; gpsimd's DGE is
                    # software, ~1us per dispatch, serialized). Unit 0's
                    # first QK chunk needs kt[0:2]+qt0[0:2]; back(0) needs V
                    # at ~6us; front(1) needs qt1 at ~7us.
                    nc.sync.dma_start(kt[:, 0:2], ktv[:, 0:2])
                    nc.sync.dma_start(qts[0][:, 0:2], qtvs[0][:, 0:2])
                    nc.sync.dma_start(kt[:, 2:4], ktv[:, 2:4])
                    nc.sync.dma_start(qts[0][:, 2:4], qtvs[0][:, 2:4])
                    nc.sync.dma_start(kt[:, 4:8], ktv[:, 4:8])
                    nc.sync.dma_start(qts[0][:, 4:8], qtvs[0][:, 4:8])
                    nc.sync.dma_start(vext[:, 0:4, 0:DV], vv[:, 0:4])
                    nc.sync.dma_start(vext[:, 4:8, 0:DV], vv[:, 4:8])
                    nc.scalar.dma_start(qts[1][:], qtvs[1])
                    # qt2/qt3 are deferred; dispatched after back(0) below
                    per_b["late0"] = (qts, qtvs)
                else:
                    nc.sync.dma_start(kt[:], ktv)
                    for h in range(HPC):
                        nc.sync.dma_start(qts[h][:], qtvs[h])
                    nc.sync.dma_start(vext[:, :, 0:DV], vv)
                nc.vector.memset(vext[:, :, DV:DV + 1], 1.0)
                per_b[b] = (kt, vext, qts)

            def emit_front_kb(b, h, pt, kb, no_dve=False):
                """QK matmuls + exp + causal mask for one k-block."""
                kt, _, qts = per_b[b]
                qt = qts[h]
                ps = pp_s.tile([128, 1024], F32, tag="ps")
                for qc in range(kb // 4, 2):
                    qs = max(128 * kb, 512 * qc)
                    qe = 512 * (qc + 1)
                    nc.tensor.matmul(
                        ps[:, qs:qe],
                        kt[:, kb, :],
                        qt[:, qs // 128:qe // 128, :],
                        start=True, stop=True, skip_group_check=True)
                if kb in DVE_EXP_KBS and not no_dve:
                    nc.vector.tensor_scalar(
                        pt[:, kb, 128 * kb:].bitcast(U16),
                        ps[:, 128 * kb:], A16 * SCALE, B16,
                        mybir.AluOpType.mult, mybir.AluOpType.add)
                else:
                    nc.scalar.activation(
                        pt[:, kb, 128 * kb:], ps[:, 128 * kb:],
                        mybir.ActivationFunctionType.Exp, scale=SCALE)
                # zero strictly-upper triangle of the diagonal block
                nc.gpsimd.affine_select(
                    out=pt[:, kb, 128 * kb:128 * (kb + 1)],
                    in_=pt[:, kb, 128 * kb:128 * (kb + 1)],
                    compare_op=mybir.AluOpType.is_ge,
                    fill=0.0, base=0,
                    pattern=[[1, 128]], channel_multiplier=-1)

            def emit_back_qb(b, h, pt, out_sb, qb):
                """PV + fused denominator for one q-block, normalize."""
                _, vext, _ = per_b[b]
                pv = pp_v.tile([128, DV + 1], F32, tag="pv")
                for kb in range(qb + 1):
                    nc.tensor.matmul(
                        pv[:],
                        pt[:, kb, 128 * qb:128 * (qb + 1)],
                        vext[:, kb, :],
                        start=(kb == 0), stop=(kb == qb),
                        skip_group_check=True)
                rinv = work.tile([128, 1], F32, tag="rinv", bufs=4)
                nc.vector.reciprocal_approx_fast(rinv[:], pv[:, DV:DV + 1])
                nc.vector.tensor_scalar(
                    out_sb[:, qb, :], pv[:, 0:DV], rinv[:], None,
                    mybir.AluOpType.mult)

            def emit_store(b, h, out_sb):
                nc.sync.dma_start(
                    out_dram[b * SEQ:(b + 1) * SEQ,
                             h * DV:(h + 1) * DV].rearrange(
                        "(qb p) d -> p qb d", p=128),
                    out_sb[:])

            units = [(b, h) for b in range(B) for h in range(HPC)]
            NU = len(units)
            emit_loads(0)
            prev = None   # (b, h, pt, out_sb)
            for u in range(NU):
                b, h = units[u]
                if h == 2 and b + 1 < B:
                    emit_loads(b + 1)   # prefetch next sequence's tiles
                pt = pt_pool.tile([128, NB, SEQ], F16, tag="pt")
                out_sb = work.tile([128, NB, DV], F16, tag="out_sb")
                last = u == NU - 1
                for kb in range(NB):
                    emit_front_kb(b, h, pt, kb)
                    if last:
                        # final slot: interleave the previous unit's back and
                        # this unit's own back (2-kb lag) between the QK
                        # k-blocks so the tail isn't one long serial drain
                        pb, ph, ppt, posb = prev
                        emit_back_qb(pb, ph, ppt, posb, kb)
                        if kb == NB - 1:
                            emit_store(pb, ph, posb)
                        if kb >= 2:
                            emit_back_qb(b, h, pt, out_sb, kb - 2)
                if not last and prev is not None:
                    pb, ph, ppt, posb = prev
                    for qb in range(NB):
                        emit_back_qb(pb, ph, ppt, posb, qb)
                    emit_store(pb, ph, posb)
                if u == 0:
                    # deferred b0 loads: dispatched once the startup-critical
                    # transfers have drained
                    qts0, qtvs0 = per_b.pop("late0")
                    nc.scalar.dma_start(qts0[2][:], qtvs0[2])
                    nc.scalar.dma_start(qts0[3][:], qtvs0[3])
                prev = (b, h, pt, out_sb)
                if last:
                    for qb in (NB - 2, NB - 1):
                        emit_back_qb(b, h, pt, out_sb, qb)
                    emit_store(b, h, out_sb)

    nc.compile()
    return nc


def run_sharded(query, key, value, trace=False):
    """Shard over 8 cores, run the bass kernel, unshard. Returns
    (out [T, H*DV] fp32, BassKernelResults)."""
    query = np.asarray(query, dtype=np.float32)
    key = np.asarray(key, dtype=np.float32)
    value = np.asarray(value, dtype=np.float32)

    if "nc" not in _BUILD_CACHE:
        _BUILD_CACHE["nc"] = _build_nc()
    nc = _BUILD_CACHE["nc"]

    # host layout glue: cast to fp16, transpose q/k to [d, t]
    qT = np.ascontiguousarray(query.astype(np.float16).T)   # [H*D, T]
    kT = np.ascontiguousarray(key.astype(np.float16).T)     # [KVH*D, T]
    v16 = np.ascontiguousarray(value.astype(np.float16))    # [T, KVH*DV]

    in_maps = []
    for c in range(NCORES):
        in_maps.append({
            "qT": np.ascontiguousarray(qT[c * HPC * D:(c + 1) * HPC * D]),
            "kT": np.ascontiguousarray(kT[c * D:(c + 1) * D]),
            "v": np.ascontiguousarray(v16[:, c * DV:(c + 1) * DV]),
        })

    res = bass_utils.run_bass_kernel_spmd(
        nc, in_maps, core_ids=list(range(NCORES)), trace=trace)

    out = np.concatenate(
        [res.results[c]["out"].astype(np.float32) for c in range(NCORES)],
        axis=1)
    return out, res


def kernel(query, key, value, seq_len=1024, **_unused):
    assert int(seq_len) == SEQ, f"kernel hardcodes seq_len={SEQ}, got {seq_len}"
    out, _ = run_sharded(query, key, value, trace=False)
    return out
